# revision 6
# baseline (speedup 1.0000x reference)
"""Trainium2 Bass kernel for nn_DendriticBranchLayer.

rate = alpha * relu(V - Vth)^2,  V = (exc + cur) / (exc + 1 + cond + inh)
  exc = x @ pruned(pre_w_exc, K=32).T        [B, OUT]
  inh = inhibitory_input @ pruned(pre_w_inh, K=16).T
  cur = sum_f branch_input.reshape(B,OUT,4)[...,f] * w_block[:,f]

Strategy: top-K selection of uniform(-2.1,-2.0) pre-weights keeps only the
top ~0.8% of the distribution, so the surviving weights all sit in a 0.12%
band and quantize to a SINGLE fp8 value — the fp8 masked-weight matrix is
exact up to one per-output-row scalar, which folds into the pointwise
constants (c_e, c_i) at zero cost.  That unlocks fp8 DoubleRow matmuls
(K=256 contraction per instruction, 2x the fp16 MAC rate): 16 exc + 8 inh
matmul instructions per (batch-block, output-block) tile instead of 32+16
fp16 ones.  x / inhibitory_input ship as fp8 (quantization error ~0.9%
rel_l2 on the final rate, gate is 2e-2); branch_input stays fp16 since the
numerator is first-order sensitive to it; output returns as fp16.

Batch dim is sharded over 8 cores.  On each core: outputs live on PSUM
partitions (128 outputs/block), batch on the free dim, so all per-output
constants (1+cond, Vth, sqrt(alpha), c_e, c_i, w_block) are per-partition
scalars fed straight into fused DVE/ACT ops.  Every DMA is a contiguous
[128, F] transfer: the host pre-swizzles all operands into the exact SBUF
tile layouts.
"""

import numpy as np
import ml_dtypes

import concourse.bass as bass
import concourse.mybir as mybir
import concourse.tile as tile
from concourse import bacc
from concourse.bass_utils import run_bass_kernel_spmd

B, OUT, EXC_IN, INH_IN, BF = 8192, 1024, 4096, 2048, 4
K_EXC, K_INH = 32, 16

NCORES = 8
BC = B // NCORES          # batch per core (1024)
P = 128                   # partitions
NB = 4                    # batch sub-blocks per core
BSUB = BC // NB           # 256 batch per sub-block
OB = OUT // P             # 8 output blocks
KE2 = EXC_IN // (2 * P)   # 16 DoubleRow k-pairs (exc)
KI2 = INH_IN // (2 * P)   # 8 DoubleRow k-pairs (inh)
KQ2 = 4                   # k-pairs in the first xt subtile (lead-in split)

# cst column layout: [P, 5*OB + OB*BF]
_C_CP1 = 0                # 1 + cond, per output
_C_VTHN = OB              # -Vth, per output
_C_SA = 2 * OB            # sqrt(alpha), per output
_C_CE = 3 * OB            # exc fp8 correction, per output
_C_CI = 4 * OB            # inh fp8 correction, per output
_C_WB = 5 * OB            # w_block[o, ob*BF + f]
_C_COLS = 5 * OB + OB * BF

F8 = ml_dtypes.float8_e4m3

_CACHE = {}
TRACE = False  # set by test harness to capture an NTFF profile


def _build_program(wb_ones):
    nc = bacc.Bacc("TRN2", target_bir_lowering=False, debug=False)
    f8, f16, f32 = mybir.dt.float8e4, mybir.dt.float16, mybir.dt.float32
    DR = mybir.MatmulPerfMode.DoubleRow

    wte = nc.declare_dram_parameter("wte", [P, OB, KE2, 2, P], f8, isOutput=False)
    wti = nc.declare_dram_parameter("wti", [P, OB, KI2, 2, P], f8, isOutput=False)
    xt = nc.declare_dram_parameter("xt", [NB, P, KE2, 2, BSUB], f8, isOutput=False)
    iht = nc.declare_dram_parameter("iht", [NB, P, KI2, 2, BSUB], f8, isOutput=False)
    brt = nc.declare_dram_parameter("brt", [NB, OB, P, BF, BSUB], f16, isOutput=False)
    cst = nc.declare_dram_parameter("cst", [P, _C_COLS], f32, isOutput=False)
    outt = nc.declare_dram_parameter("outt", [OB, P, NB, BSUB], f16, isOutput=True)

    add = mybir.AluOpType.add
    mult = mybir.AluOpType.mult
    Relu = mybir.ActivationFunctionType.Relu
    Square = mybir.ActivationFunctionType.Square
    Identity = mybir.ActivationFunctionType.Identity

    with tile.TileContext(nc) as tc:
        with tc.tile_pool(name="wpool", bufs=1) as wpool, \
             tc.tile_pool(name="xpool", bufs=2) as xpool, \
             tc.tile_pool(name="ipool", bufs=2) as ipool, \
             tc.tile_pool(name="brpool", bufs=4) as brpool, \
             tc.tile_pool(name="wk", bufs=3) as wk, \
             tc.tile_pool(name="wk2", bufs=1) as wk2, \
             tc.tile_pool(name="ps_exc", bufs=4, space="PSUM") as ps_exc, \
             tc.tile_pool(name="ps_inh", bufs=4, space="PSUM") as ps_inh:

            cst_s = wpool.tile([P, _C_COLS], f32)
            # per-ob weight tiles, loaded in first-use order (ahead of need)
            wte_sb, wti_sb = [None] * OB, [None] * OB

            def load_weights(ob):
                if ob >= OB or wte_sb[ob] is not None:
                    return
                we = wpool.tile([P, KE2, 2, P], f8, tag=f"wte{ob}")
                nc.sync.dma_start(out=we, in_=wte[:, ob, :, :, :])
                wte_sb[ob] = we
                wi = wpool.tile([P, KI2, 2, P], f8, tag=f"wti{ob}")
                nc.sync.dma_start(out=wi, in_=wti[:, ob, :, :, :])
                wti_sb[ob] = wi

            # critical lead-in: split the first tiles so the first inh chain
            # can start after ~190 KB instead of 750 KB of DMA.
            wi0 = wpool.tile([P, KI2, 2, P], f8, tag="wti0")
            nc.sync.dma_start(out=wi0[:, 0:2, :, :], in_=wti[:, 0, 0:2, :, :])
            wti_sb[0] = wi0

            xi_tiles = {}

            def load_nb(nb):
                if nb >= NB or nb in xi_tiles:
                    return
                xsa = xpool.tile([P, KQ2, 2, BSUB], f8, tag="xta")
                nc.sync.dma_start(out=xsa, in_=xt[nb, :, 0:KQ2, :, :])
                xsb = xpool.tile([P, KE2 - KQ2, 2, BSUB], f8, tag="xtb")
                nc.sync.dma_start(out=xsb, in_=xt[nb, :, KQ2:KE2, :, :])
                xs = (xsa, xsb)
                ihs = ipool.tile([P, KI2, 2, BSUB], f8, tag="iht")
                nc.sync.dma_start(out=ihs, in_=iht[nb, :, :, :, :])
                xi_tiles[nb] = (xs, ihs)

            ihs0 = ipool.tile([P, KI2, 2, BSUB], f8, tag="iht")
            nc.sync.dma_start(out=ihs0[:, 0:2, :, :], in_=iht[0, :, 0:2, :, :])
            nc.sync.dma_start(out=wi0[:, 2:KI2, :, :], in_=wti[:, 0, 2:KI2, :, :])
            nc.sync.dma_start(out=ihs0[:, 2:KI2, :, :], in_=iht[0, :, 2:KI2, :, :])
            we0 = wpool.tile([P, KE2, 2, P], f8, tag="wte0")
            nc.sync.dma_start(out=we0, in_=wte[:, 0, :, :, :])
            wte_sb[0] = we0
            xsa0 = xpool.tile([P, KQ2, 2, BSUB], f8, tag="xta")
            nc.sync.dma_start(out=xsa0, in_=xt[0, :, 0:KQ2, :, :])
            xsb0 = xpool.tile([P, KE2 - KQ2, 2, BSUB], f8, tag="xtb")
            nc.sync.dma_start(out=xsb0, in_=xt[0, :, KQ2:KE2, :, :])
            xi_tiles[0] = ((xsa0, xsb0), ihs0)
            nc.sync.dma_start(out=cst_s, in_=cst[:, :])

            for nb in range(NB):
                xt_s, iht_s = xi_tiles[nb]

                for ob in range(OB):
                    br_s = brpool.tile([P, BF, BSUB], f16, tag="br")
                    nc.sync.dma_start(out=br_s, in_=brt[nb, ob, :, :, :])
                    if nb == 0:
                        # one weight pair ~2 iterations ahead: keeps the head
                        # DMA queue clear for the critical first x/ih tiles
                        load_weights(ob + 1 if ob == 0 else ob + 2)
                        if ob == 0:
                            load_weights(2)
                    if ob == OB - 5:
                        load_nb(nb + 1)

                    exc_ps = ps_exc.tile([P, BSUB], f32, tag="exc")
                    inh_ps = ps_inh.tile([P, BSUB], f32, tag="inh")

                    def emit_inh():
                        for k in range(KI2):
                            nc.tensor.matmul(
                                inh_ps, wti_sb[ob][:, k, :, :], iht_s[:, k, :, :],
                                start=(k == 0), stop=(k == KI2 - 1), perf_mode=DR)

                    def emit_exc():
                        xsa, xsb = xt_s
                        for k in range(KE2):
                            rhs = (xsa[:, k, :, :] if k < KQ2
                                   else xsb[:, k - KQ2, :, :])
                            nc.tensor.matmul(
                                exc_ps, wte_sb[ob][:, k, :, :], rhs,
                                start=(k == 0), stop=(k == KE2 - 1), perf_mode=DR)

                    last2 = nb == NB - 1 and ob >= OB - 2
                    if (nb == 0 and ob == 0) or last2:
                        # first iter: inh data lands first; last iters: finish
                        # inh early so the pointwise chain only waits on exc
                        emit_inh()
                        emit_exc()
                    else:
                        emit_exc()
                        emit_inh()

                    def pointwise(pool, c0, w, sfx):
                        cs = slice(c0, c0 + w)
                        ce_col = cst_s[:, _C_CE + ob: _C_CE + ob + 1]
                        ci_col = cst_s[:, _C_CI + ob: _C_CI + ob + 1]
                        # cur = sum_f br[:, f, cs] * w_block[o, f]  (GpSimd)
                        if wb_ones:
                            t0 = pool.tile([P, w], f32, tag="cur0" + sfx)
                            nc.gpsimd.tensor_add(t0, br_s[:, 0, cs], br_s[:, 1, cs])
                            t1 = pool.tile([P, w], f32, tag="cur1" + sfx)
                            nc.gpsimd.tensor_add(t1, br_s[:, 2, cs], br_s[:, 3, cs])
                            cur = pool.tile([P, w], f32, tag="cur" + sfx)
                            nc.gpsimd.tensor_add(cur, t0, t1)
                        else:
                            cur = pool.tile([P, w], f32, tag="cur" + sfx)
                            nc.gpsimd.tensor_scalar_mul(
                                cur, br_s[:, 0, cs],
                                cst_s[:, _C_WB + ob * BF: _C_WB + ob * BF + 1])
                            for f in range(1, BF):
                                nxt = pool.tile([P, w], f32, tag=f"cur{f % 2}" + sfx)
                                nc.gpsimd.scalar_tensor_tensor(
                                    nxt, br_s[:, f, cs],
                                    cst_s[:, _C_WB + ob * BF + f: _C_WB + ob * BF + f + 1],
                                    cur, op0=mult, op1=add)
                                cur = nxt

                        # num = c_e * exc + cur   (one fused DVE op)
                        num = pool.tile([P, w], f32, tag="num" + sfx)
                        nc.vector.scalar_tensor_tensor(
                            num, exc_ps[:, cs], ce_col, cur, op0=mult, op1=add)
                        # exc1 = c_e * exc + (1 + cond) on ACT
                        exc1 = pool.tile([P, w], f32, tag="exc1" + sfx)
                        nc.scalar.activation(
                            exc1, exc_ps[:, cs], Identity,
                            bias=cst_s[:, _C_CP1 + ob: _C_CP1 + ob + 1],
                            scale=ce_col)
                        # den = c_i * inh + exc1  (one fused DVE op)
                        den = pool.tile([P, w], f32, tag="den" + sfx)
                        nc.vector.scalar_tensor_tensor(
                            den, inh_ps[:, cs], ci_col, exc1, op0=mult, op1=add)
                        rden = pool.tile([P, w], f32, tag="rden" + sfx)
                        nc.vector.reciprocal_approx_fast(rden, den)
                        v = pool.tile([P, w], f32, tag="v" + sfx)
                        nc.vector.tensor_mul(v, num, rden)
                        # r = relu(v - Vth); rate = (r * sqrt(alpha))^2
                        r = pool.tile([P, w], f32, tag="r" + sfx)
                        nc.scalar.activation(
                            r, v, Relu, bias=cst_s[:, _C_VTHN + ob: _C_VTHN + ob + 1])
                        ot = pool.tile([P, w], f16, tag="ot" + sfx)
                        nc.scalar.activation(
                            ot, r, Square, scale=cst_s[:, _C_SA + ob: _C_SA + ob + 1])
                        nc.sync.dma_start(out=outt[ob, :, nb, cs], in_=ot)

                    if nb == NB - 1 and ob == OB - 1:
                        # split the final chain so the kernel tail is shorter
                        q = BSUB // 4
                        for h in range(4):
                            pointwise(wk2, h * q, q, f"q{h}")
                    elif nb == NB - 1 and ob == OB - 2:
                        pointwise(wk2, 0, BSUB // 2, "h0")
                        pointwise(wk2, BSUB // 2, BSUB // 2, "h1")
                    else:
                        pointwise(wk, 0, BSUB, "")

    nc.compile()
    return nc


def _pruned_dense(pre_w, K):
    """Masked weight [out, in] fp32. Tie-break matches jax.lax.top_k:
    equal values -> lower index wins (stable sort)."""
    idx = np.argsort(-pre_w, axis=1, kind="stable")[:, :K]
    w = np.exp(pre_w.astype(np.float32))
    dense = np.zeros(pre_w.shape, dtype=np.float32)
    np.put_along_axis(dense, idx, np.take_along_axis(w, idx, axis=1), axis=1)
    return dense


def _quant_w(dense):
    """fp8 weights + per-output-row lsq correction c (dequant scale)."""
    q8 = dense.astype(F8)
    dq = q8.astype(np.float32)
    num = (dq * dense).sum(axis=1)
    den = (dq * dq).sum(axis=1)
    c = np.where(den > 0, num / np.maximum(den, 1e-30), 1.0).astype(np.float32)
    return q8, c


def kernel(x, inhibitory_input, branch_input, pre_w_exc, pre_w_inh,
           w_block, presigmoid_Vth, log_alpha_max):
    w_block = np.asarray(w_block, dtype=np.float32)
    wb_ones = bool(np.all(w_block == 1.0))
    key = ("nc", wb_ones)
    if key not in _CACHE:
        _CACHE[key] = _build_program(wb_ones)
    nc = _CACHE[key]

    x = np.ascontiguousarray(np.asarray(x, dtype=np.float32))
    inh = np.ascontiguousarray(np.asarray(inhibitory_input, dtype=np.float32))
    br = np.ascontiguousarray(np.asarray(branch_input, dtype=np.float32))
    pre_w_exc = np.asarray(pre_w_exc, dtype=np.float32)
    pre_w_inh = np.asarray(pre_w_inh, dtype=np.float32)
    presigmoid_Vth = np.asarray(presigmoid_Vth, dtype=np.float32)
    log_alpha_max = np.asarray(log_alpha_max, dtype=np.float32)

    # --- replicated operands -------------------------------------------------
    we8, ce = _quant_w(_pruned_dense(pre_w_exc, K_EXC))   # [OUT, EXC_IN] fp8
    wi8, ci = _quant_w(_pruned_dense(pre_w_inh, K_INH))   # [OUT, INH_IN] fp8
    # wte[p, ob, kk, j, o] = W8_exc[ob*P + o, (2*kk + j)*P + p]
    wte = np.ascontiguousarray(
        we8.T.reshape(KE2, 2, P, OB, P).transpose(2, 3, 0, 1, 4))
    wti = np.ascontiguousarray(
        wi8.T.reshape(KI2, 2, P, OB, P).transpose(2, 3, 0, 1, 4))

    cond = w_block.sum(axis=1, dtype=np.float32)              # [OUT]
    vth = (1.0 / (1.0 + np.exp(-presigmoid_Vth.astype(np.float64)))).astype(np.float32)
    sa = np.sqrt(np.exp(log_alpha_max.astype(np.float32)))
    cst = np.zeros((P, _C_COLS), dtype=np.float32)
    cst[:, _C_CP1:_C_CP1 + OB] = (1.0 + cond).reshape(OB, P).T
    cst[:, _C_VTHN:_C_VTHN + OB] = (-vth).reshape(OB, P).T
    cst[:, _C_SA:_C_SA + OB] = sa.reshape(OB, P).T
    cst[:, _C_CE:_C_CE + OB] = ce.reshape(OB, P).T
    cst[:, _C_CI:_C_CI + OB] = ci.reshape(OB, P).T
    cst[:, _C_WB:] = w_block.reshape(OB, P, BF).transpose(1, 0, 2).reshape(P, OB * BF)

    # --- per-core shards -----------------------------------------------------
    in_maps = []
    for c in range(NCORES):
        s = slice(c * BC, (c + 1) * BC)
        # xt[nb, p, kk, j, b] = x[c*BC + nb*BSUB + b, (2*kk + j)*P + p]
        xt = np.ascontiguousarray(
            x[s].astype(F8).reshape(NB, BSUB, KE2, 2, P).transpose(0, 4, 2, 3, 1))
        iht = np.ascontiguousarray(
            inh[s].astype(F8).reshape(NB, BSUB, KI2, 2, P).transpose(0, 4, 2, 3, 1))
        # brt[nb, ob, o, f, b] = branch[c*BC + nb*BSUB + b, (ob*P + o)*BF + f]
        brt = np.ascontiguousarray(
            br[s].astype(np.float16).reshape(NB, BSUB, OB, P, BF).transpose(0, 2, 3, 4, 1))
        in_maps.append({"wte": wte, "wti": wti, "cst": cst,
                        "xt": xt, "iht": iht, "brt": brt})

    res = run_bass_kernel_spmd(nc, in_maps, list(range(NCORES)), trace=TRACE)
    _CACHE["last"] = res

    out = np.empty((B, OUT), dtype=np.float32)
    for c in range(NCORES):
        # outt[ob, o, nb, b] -> out[c*BC + nb*BSUB + b, ob*P + o]
        ot = res.results[c]["outt"].astype(np.float32)
        out[c * BC:(c + 1) * BC] = ot.transpose(2, 3, 0, 1).reshape(BC, OUT)
    return out


# revision 9
# speedup vs baseline: 1.0955x; 1.0955x over previous
"""Trainium2 Bass kernel for nn_DendriticBranchLayer.

rate = alpha * relu(V - Vth)^2,  V = (exc + cur) / (exc + 1 + cond + inh)
  exc = x @ pruned(pre_w_exc, K=32).T        [B, OUT]
  inh = inhibitory_input @ pruned(pre_w_inh, K=16).T
  cur = sum_f branch_input.reshape(B,OUT,4)[...,f] * w_block[:,f]

Strategy: top-K selection of uniform(-2.1,-2.0) pre-weights keeps only the
top ~0.8% of the distribution, so the surviving weights all sit in a 0.12%
band and quantize to a SINGLE fp8 value — the fp8 masked-weight matrix is
exact up to one per-output-row scalar, which folds into the pointwise
constants (c_e, c_i) at zero cost.  That unlocks fp8 DoubleRow matmuls
(K=256 contraction per instruction, 2x the fp16 MAC rate): 16 exc + 8 inh
matmul instructions per (batch-block, output-block) tile instead of 32+16
fp16 ones.  x / inhibitory_input ship as fp8 (quantization error ~0.9%
rel_l2 on the final rate, gate is 2e-2); branch_input stays fp16 since the
numerator is first-order sensitive to it; output returns as fp16.

Batch dim is sharded over 8 cores.  On each core: outputs live on PSUM
partitions (128 outputs/block), batch on the free dim, so all per-output
constants (1+cond, Vth, sqrt(alpha), c_e, c_i, w_block) are per-partition
scalars fed straight into fused DVE/ACT ops.  Every DMA is a contiguous
[128, F] transfer: the host pre-swizzles all operands into the exact SBUF
tile layouts.
"""

import numpy as np
import ml_dtypes

import concourse.bass as bass
import concourse.mybir as mybir
import concourse.tile as tile
from concourse import bacc
from concourse.bass_utils import run_bass_kernel_spmd

B, OUT, EXC_IN, INH_IN, BF = 8192, 1024, 4096, 2048, 4
K_EXC, K_INH = 32, 16

NCORES = 8
BC = B // NCORES          # batch per core (1024)
P = 128                   # partitions
NB = 4                    # batch sub-blocks per core
BSUB = BC // NB           # 256 batch per sub-block
OB = OUT // P             # 8 output blocks
KE2 = EXC_IN // (2 * P)   # 16 DoubleRow k-pairs (exc)
KI2 = INH_IN // (2 * P)   # 8 DoubleRow k-pairs (inh)
KQ2 = 4                   # k-pairs in the first xt subtile (lead-in split)

# cst column layout: [P, 5*OB + OB*BF]
_C_CP1 = 0                # 1 + cond, per output
_C_VTHN = OB              # -Vth, per output
_C_SA = 2 * OB            # sqrt(alpha), per output
_C_CE = 3 * OB            # exc fp8 correction, per output
_C_CI = 4 * OB            # inh fp8 correction, per output
_C_WB = 5 * OB            # w_block[o, ob*BF + f]
_C_COLS = 5 * OB + OB * BF

F8 = ml_dtypes.float8_e4m3

_CACHE = {}
TRACE = False  # set by test harness to capture an NTFF profile


def _build_program(wb_ones):
    nc = bacc.Bacc("TRN2", target_bir_lowering=False, debug=False)
    f8, f16, f32 = mybir.dt.float8e4, mybir.dt.float16, mybir.dt.float32
    DR = mybir.MatmulPerfMode.DoubleRow

    wte = nc.declare_dram_parameter("wte", [P, OB, KE2, 2, P], f8, isOutput=False)
    wti = nc.declare_dram_parameter("wti", [P, OB, KI2, 2, P], f8, isOutput=False)
    xt = nc.declare_dram_parameter("xt", [NB, P, KE2, 2, BSUB], f8, isOutput=False)
    iht = nc.declare_dram_parameter("iht", [NB, P, KI2, 2, BSUB], f8, isOutput=False)
    brt = nc.declare_dram_parameter("brt", [NB, OB, P, BF, BSUB], f16, isOutput=False)
    cst = nc.declare_dram_parameter("cst", [P, _C_COLS], f32, isOutput=False)
    outt = nc.declare_dram_parameter("outt", [OB, P, NB, BSUB], f16, isOutput=True)

    add = mybir.AluOpType.add
    mult = mybir.AluOpType.mult
    Relu = mybir.ActivationFunctionType.Relu
    Square = mybir.ActivationFunctionType.Square
    Identity = mybir.ActivationFunctionType.Identity

    with tile.TileContext(nc) as tc:
        with tc.tile_pool(name="wpool", bufs=1) as wpool, \
             tc.tile_pool(name="xpool", bufs=2) as xpool, \
             tc.tile_pool(name="ipool", bufs=2) as ipool, \
             tc.tile_pool(name="brpool", bufs=6) as brpool, \
             tc.tile_pool(name="wk", bufs=4) as wk, \
             tc.tile_pool(name="wk2", bufs=1) as wk2, \
             tc.tile_pool(name="ps_exc", bufs=4, space="PSUM") as ps_exc, \
             tc.tile_pool(name="ps_inh", bufs=4, space="PSUM") as ps_inh:

            cst_s = wpool.tile([P, _C_COLS], f32)
            # per-ob weight tiles, loaded in first-use order (ahead of need)
            wte_sb, wti_sb = [None] * OB, [None] * OB

            def load_weights(ob):
                # issued on the scalar (Activation) HW-DGE queue: parallel to
                # the sync queue carrying the critical x/ih/br tiles
                if ob >= OB or wte_sb[ob] is not None:
                    return
                we = wpool.tile([P, KE2, 2, P], f8, tag=f"wte{ob}")
                nc.scalar.dma_start(out=we, in_=wte[:, ob, :, :, :])
                wte_sb[ob] = we
                wi = wpool.tile([P, KI2, 2, P], f8, tag=f"wti{ob}")
                nc.scalar.dma_start(out=wi, in_=wti[:, ob, :, :, :])
                wti_sb[ob] = wi

            # critical lead-in: split the first tiles so the first inh chain
            # can start after ~190 KB instead of 750 KB of DMA.
            wi0 = wpool.tile([P, KI2, 2, P], f8, tag="wti0")
            nc.sync.dma_start(out=wi0[:, 0:2, :, :], in_=wti[:, 0, 0:2, :, :])
            wti_sb[0] = wi0

            xi_tiles = {}

            def load_nb(nb):
                if nb >= NB or nb in xi_tiles:
                    return
                xsa = xpool.tile([P, KQ2, 2, BSUB], f8, tag="xta")
                nc.sync.dma_start(out=xsa, in_=xt[nb, :, 0:KQ2, :, :])
                xsb = xpool.tile([P, KE2 - KQ2, 2, BSUB], f8, tag="xtb")
                nc.sync.dma_start(out=xsb, in_=xt[nb, :, KQ2:KE2, :, :])
                xs = (xsa, xsb)
                ihs = ipool.tile([P, KI2, 2, BSUB], f8, tag="iht")
                nc.sync.dma_start(out=ihs, in_=iht[nb, :, :, :, :])
                xi_tiles[nb] = (xs, ihs)

            ihs0 = ipool.tile([P, KI2, 2, BSUB], f8, tag="iht")
            nc.sync.dma_start(out=ihs0[:, 0:2, :, :], in_=iht[0, :, 0:2, :, :])
            nc.sync.dma_start(out=wi0[:, 2:KI2, :, :], in_=wti[:, 0, 2:KI2, :, :])
            nc.sync.dma_start(out=ihs0[:, 2:KI2, :, :], in_=iht[0, :, 2:KI2, :, :])
            we0 = wpool.tile([P, KE2, 2, P], f8, tag="wte0")
            nc.sync.dma_start(out=we0, in_=wte[:, 0, :, :, :])
            wte_sb[0] = we0
            xsa0 = xpool.tile([P, KQ2, 2, BSUB], f8, tag="xta")
            nc.sync.dma_start(out=xsa0, in_=xt[0, :, 0:KQ2, :, :])
            xsb0 = xpool.tile([P, KE2 - KQ2, 2, BSUB], f8, tag="xtb")
            nc.sync.dma_start(out=xsb0, in_=xt[0, :, KQ2:KE2, :, :])
            xi_tiles[0] = ((xsa0, xsb0), ihs0)
            nc.sync.dma_start(out=cst_s, in_=cst[:, :])

            for nb in range(NB):
                xt_s, iht_s = xi_tiles[nb]

                for ob in range(OB):
                    br_s = brpool.tile([P, BF, BSUB], f16, tag="br")
                    nc.sync.dma_start(out=br_s, in_=brt[nb, ob, :, :, :])
                    if nb == 0:
                        for ahead in (1, 2, 3, 4):
                            load_weights(ob + ahead)
                    if ob == OB - 5:
                        load_nb(nb + 1)

                    exc_ps = ps_exc.tile([P, BSUB], f32, tag="exc")
                    inh_ps = ps_inh.tile([P, BSUB], f32, tag="inh")

                    def emit_inh():
                        for k in range(KI2):
                            nc.tensor.matmul(
                                inh_ps, wti_sb[ob][:, k, :, :], iht_s[:, k, :, :],
                                start=(k == 0), stop=(k == KI2 - 1), perf_mode=DR)

                    def emit_exc():
                        xsa, xsb = xt_s
                        for k in range(KE2):
                            rhs = (xsa[:, k, :, :] if k < KQ2
                                   else xsb[:, k - KQ2, :, :])
                            nc.tensor.matmul(
                                exc_ps, wte_sb[ob][:, k, :, :], rhs,
                                start=(k == 0), stop=(k == KE2 - 1), perf_mode=DR)

                    last2 = nb == NB - 1 and ob >= OB - 2
                    if (nb == 0 and ob == 0) or last2:
                        # first iter: inh data lands first; last iters: finish
                        # inh early so the pointwise chain only waits on exc
                        emit_inh()
                        emit_exc()
                    else:
                        emit_exc()
                        emit_inh()

                    def pointwise(pool, c0, w, sfx):
                        cs = slice(c0, c0 + w)
                        ce_col = cst_s[:, _C_CE + ob: _C_CE + ob + 1]
                        ci_col = cst_s[:, _C_CI + ob: _C_CI + ob + 1]
                        # cur = sum_f br[:, f, cs] * w_block[o, f]  (GpSimd)
                        if wb_ones:
                            t0 = pool.tile([P, w], f32, tag="cur0" + sfx)
                            nc.gpsimd.tensor_add(t0, br_s[:, 0, cs], br_s[:, 1, cs])
                            t1 = pool.tile([P, w], f32, tag="cur1" + sfx)
                            nc.gpsimd.tensor_add(t1, br_s[:, 2, cs], br_s[:, 3, cs])
                            cur = pool.tile([P, w], f32, tag="cur" + sfx)
                            nc.gpsimd.tensor_add(cur, t0, t1)
                        else:
                            cur = pool.tile([P, w], f32, tag="cur" + sfx)
                            nc.gpsimd.tensor_scalar_mul(
                                cur, br_s[:, 0, cs],
                                cst_s[:, _C_WB + ob * BF: _C_WB + ob * BF + 1])
                            for f in range(1, BF):
                                nxt = pool.tile([P, w], f32, tag=f"cur{f % 2}" + sfx)
                                nc.gpsimd.scalar_tensor_tensor(
                                    nxt, br_s[:, f, cs],
                                    cst_s[:, _C_WB + ob * BF + f: _C_WB + ob * BF + f + 1],
                                    cur, op0=mult, op1=add)
                                cur = nxt

                        # num = c_e * exc + cur   (one fused DVE op)
                        num = pool.tile([P, w], f32, tag="num" + sfx)
                        nc.vector.scalar_tensor_tensor(
                            num, exc_ps[:, cs], ce_col, cur, op0=mult, op1=add)
                        # exc1 = c_e * exc + (1 + cond) on ACT
                        exc1 = pool.tile([P, w], f32, tag="exc1" + sfx)
                        nc.scalar.activation(
                            exc1, exc_ps[:, cs], Identity,
                            bias=cst_s[:, _C_CP1 + ob: _C_CP1 + ob + 1],
                            scale=ce_col)
                        # den = c_i * inh + exc1  (one fused DVE op)
                        den = pool.tile([P, w], f32, tag="den" + sfx)
                        nc.vector.scalar_tensor_tensor(
                            den, inh_ps[:, cs], ci_col, exc1, op0=mult, op1=add)
                        rden = pool.tile([P, w], f32, tag="rden" + sfx)
                        nc.vector.reciprocal_approx_fast(rden, den)
                        v = pool.tile([P, w], f32, tag="v" + sfx)
                        nc.vector.tensor_mul(v, num, rden)
                        # r = relu(v - Vth); rate = (r * sqrt(alpha))^2
                        r = pool.tile([P, w], f32, tag="r" + sfx)
                        nc.scalar.activation(
                            r, v, Relu, bias=cst_s[:, _C_VTHN + ob: _C_VTHN + ob + 1])
                        ot = pool.tile([P, w], f16, tag="ot" + sfx)
                        nc.scalar.activation(
                            ot, r, Square, scale=cst_s[:, _C_SA + ob: _C_SA + ob + 1])
                        nc.sync.dma_start(out=outt[ob, :, nb, cs], in_=ot)

                    if nb == NB - 1 and ob == OB - 1:
                        # split the final chain so the kernel tail is shorter
                        q = BSUB // 4
                        for h in range(4):
                            pointwise(wk2, h * q, q, f"q{h}")
                    elif nb == NB - 1 and ob == OB - 2:
                        pointwise(wk2, 0, BSUB // 2, "h0")
                        pointwise(wk2, BSUB // 2, BSUB // 2, "h1")
                    else:
                        pointwise(wk, 0, BSUB, "")

    nc.compile()
    return nc


def _pruned_dense(pre_w, K):
    """Masked weight [out, in] fp32. Tie-break matches jax.lax.top_k:
    equal values -> lower index wins (stable sort)."""
    idx = np.argsort(-pre_w, axis=1, kind="stable")[:, :K]
    w = np.exp(pre_w.astype(np.float32))
    dense = np.zeros(pre_w.shape, dtype=np.float32)
    np.put_along_axis(dense, idx, np.take_along_axis(w, idx, axis=1), axis=1)
    return dense


def _quant_w(dense):
    """fp8 weights + per-output-row lsq correction c (dequant scale)."""
    q8 = dense.astype(F8)
    dq = q8.astype(np.float32)
    num = (dq * dense).sum(axis=1)
    den = (dq * dq).sum(axis=1)
    c = np.where(den > 0, num / np.maximum(den, 1e-30), 1.0).astype(np.float32)
    return q8, c


def kernel(x, inhibitory_input, branch_input, pre_w_exc, pre_w_inh,
           w_block, presigmoid_Vth, log_alpha_max):
    w_block = np.asarray(w_block, dtype=np.float32)
    wb_ones = bool(np.all(w_block == 1.0))
    key = ("nc", wb_ones)
    if key not in _CACHE:
        _CACHE[key] = _build_program(wb_ones)
    nc = _CACHE[key]

    x = np.ascontiguousarray(np.asarray(x, dtype=np.float32))
    inh = np.ascontiguousarray(np.asarray(inhibitory_input, dtype=np.float32))
    br = np.ascontiguousarray(np.asarray(branch_input, dtype=np.float32))
    pre_w_exc = np.asarray(pre_w_exc, dtype=np.float32)
    pre_w_inh = np.asarray(pre_w_inh, dtype=np.float32)
    presigmoid_Vth = np.asarray(presigmoid_Vth, dtype=np.float32)
    log_alpha_max = np.asarray(log_alpha_max, dtype=np.float32)

    # --- replicated operands -------------------------------------------------
    we8, ce = _quant_w(_pruned_dense(pre_w_exc, K_EXC))   # [OUT, EXC_IN] fp8
    wi8, ci = _quant_w(_pruned_dense(pre_w_inh, K_INH))   # [OUT, INH_IN] fp8
    # wte[p, ob, kk, j, o] = W8_exc[ob*P + o, (2*kk + j)*P + p]
    wte = np.ascontiguousarray(
        we8.T.reshape(KE2, 2, P, OB, P).transpose(2, 3, 0, 1, 4))
    wti = np.ascontiguousarray(
        wi8.T.reshape(KI2, 2, P, OB, P).transpose(2, 3, 0, 1, 4))

    cond = w_block.sum(axis=1, dtype=np.float32)              # [OUT]
    vth = (1.0 / (1.0 + np.exp(-presigmoid_Vth.astype(np.float64)))).astype(np.float32)
    sa = np.sqrt(np.exp(log_alpha_max.astype(np.float32)))
    cst = np.zeros((P, _C_COLS), dtype=np.float32)
    cst[:, _C_CP1:_C_CP1 + OB] = (1.0 + cond).reshape(OB, P).T
    cst[:, _C_VTHN:_C_VTHN + OB] = (-vth).reshape(OB, P).T
    cst[:, _C_SA:_C_SA + OB] = sa.reshape(OB, P).T
    cst[:, _C_CE:_C_CE + OB] = ce.reshape(OB, P).T
    cst[:, _C_CI:_C_CI + OB] = ci.reshape(OB, P).T
    cst[:, _C_WB:] = w_block.reshape(OB, P, BF).transpose(1, 0, 2).reshape(P, OB * BF)

    # --- per-core shards -----------------------------------------------------
    in_maps = []
    for c in range(NCORES):
        s = slice(c * BC, (c + 1) * BC)
        # xt[nb, p, kk, j, b] = x[c*BC + nb*BSUB + b, (2*kk + j)*P + p]
        xt = np.ascontiguousarray(
            x[s].astype(F8).reshape(NB, BSUB, KE2, 2, P).transpose(0, 4, 2, 3, 1))
        iht = np.ascontiguousarray(
            inh[s].astype(F8).reshape(NB, BSUB, KI2, 2, P).transpose(0, 4, 2, 3, 1))
        # brt[nb, ob, o, f, b] = branch[c*BC + nb*BSUB + b, (ob*P + o)*BF + f]
        brt = np.ascontiguousarray(
            br[s].astype(np.float16).reshape(NB, BSUB, OB, P, BF).transpose(0, 2, 3, 4, 1))
        in_maps.append({"wte": wte, "wti": wti, "cst": cst,
                        "xt": xt, "iht": iht, "brt": brt})

    res = run_bass_kernel_spmd(nc, in_maps, list(range(NCORES)), trace=TRACE)
    _CACHE["last"] = res

    out = np.empty((B, OUT), dtype=np.float32)
    for c in range(NCORES):
        # outt[ob, o, nb, b] -> out[c*BC + nb*BSUB + b, ob*P + o]
        ot = res.results[c]["outt"].astype(np.float32)
        out[c * BC:(c + 1) * BC] = ot.transpose(2, 3, 0, 1).reshape(BC, OUT)
    return out


# revision 16
# speedup vs baseline: 1.1674x; 1.0657x over previous
"""Trainium2 Bass kernel for nn_DendriticBranchLayer.

rate = alpha * relu(V - Vth)^2,  V = (exc + cur) / (exc + 1 + cond + inh)
  exc = x @ pruned(pre_w_exc, K=32).T        [B, OUT]
  inh = inhibitory_input @ pruned(pre_w_inh, K=16).T
  cur = sum_f branch_input.reshape(B,OUT,4)[...,f] * w_block[:,f]

Strategy: top-K selection of uniform(-2.1,-2.0) pre-weights keeps only the
top ~0.8% of the distribution, so the surviving weights all sit in a 0.12%
band and quantize to a SINGLE fp8 value — the fp8 masked-weight matrix is
exact up to one per-output-row scalar, which folds into the pointwise
constants (c_e, c_i) at zero cost.  That unlocks fp8 DoubleRow matmuls
(K=256 contraction per instruction, 2x the fp16 MAC rate): 16 exc + 8 inh
matmul instructions per (batch-block, output-block) tile instead of 32+16
fp16 ones.  x / inhibitory_input ship as fp8 (quantization error ~0.9%
rel_l2 on the final rate, gate is 2e-2); branch_input stays fp16 since the
numerator is first-order sensitive to it; output returns as fp16.

Batch dim is sharded over 8 cores.  On each core: outputs live on PSUM
partitions (128 outputs/block), batch on the free dim, so all per-output
constants (1+cond, Vth, sqrt(alpha), c_e, c_i, w_block) are per-partition
scalars fed straight into fused DVE/ACT ops.  Every DMA is a contiguous
[128, F] transfer: the host pre-swizzles all operands into the exact SBUF
tile layouts.
"""

import numpy as np
import ml_dtypes

import concourse.bass as bass
import concourse.mybir as mybir
import concourse.tile as tile
from concourse import bacc
from concourse.bass_utils import run_bass_kernel_spmd

B, OUT, EXC_IN, INH_IN, BF = 8192, 1024, 4096, 2048, 4
K_EXC, K_INH = 32, 16

NCORES = 8
BC = B // NCORES          # batch per core (1024)
P = 128                   # partitions
NB = 4                    # batch sub-blocks per core
BSUB = BC // NB           # 256 batch per sub-block
OB = OUT // P             # 8 output blocks
KE2 = EXC_IN // (2 * P)   # 16 DoubleRow k-pairs (exc)
KI2 = INH_IN // (2 * P)   # 8 DoubleRow k-pairs (inh)
KQ2 = 4                   # k-pairs in the first xt subtile (lead-in split)

# cst column layout: [P, 5*OB + OB*BF]
_C_CP1 = 0                # 1 + cond, per output
_C_VTHN = OB              # -Vth, per output
_C_SA = 2 * OB            # sqrt(alpha), per output
_C_CE = 3 * OB            # exc fp8 correction, per output
_C_CI = 4 * OB            # inh fp8 correction, per output
_C_WB = 5 * OB            # w_block[o, ob*BF + f]
_C_COLS = 5 * OB + OB * BF

F8 = ml_dtypes.float8_e4m3

_CACHE = {}
TRACE = False  # set by test harness to capture an NTFF profile


def _build_program(wb_ones):
    nc = bacc.Bacc("TRN2", target_bir_lowering=False, debug=False)
    f8, f16, f32 = mybir.dt.float8e4, mybir.dt.float16, mybir.dt.float32
    DR = mybir.MatmulPerfMode.DoubleRow

    wte = nc.declare_dram_parameter("wte", [P, OB, KE2, 2, P], f8, isOutput=False)
    wti = nc.declare_dram_parameter("wti", [P, OB, KI2, 2, P], f8, isOutput=False)
    xt = nc.declare_dram_parameter("xt", [NB, P, KE2, 2, BSUB], f8, isOutput=False)
    iht = nc.declare_dram_parameter("iht", [NB, P, KI2, 2, BSUB], f8, isOutput=False)
    brt = nc.declare_dram_parameter("brt", [NB, OB, P, BSUB, BF], f16, isOutput=False)
    cst = nc.declare_dram_parameter("cst", [P, _C_COLS], f32, isOutput=False)
    outt = nc.declare_dram_parameter("outt", [OB, P, NB, BSUB], f16, isOutput=True)

    add = mybir.AluOpType.add
    mult = mybir.AluOpType.mult
    Relu = mybir.ActivationFunctionType.Relu
    Square = mybir.ActivationFunctionType.Square
    Identity = mybir.ActivationFunctionType.Identity
    AxX = mybir.AxisListType.X

    with tile.TileContext(nc) as tc:
        with tc.tile_pool(name="wpool", bufs=1) as wpool, \
             tc.tile_pool(name="xpool", bufs=2) as xpool, \
             tc.tile_pool(name="ipool", bufs=2) as ipool, \
             tc.tile_pool(name="brpool", bufs=6) as brpool, \
             tc.tile_pool(name="wk", bufs=4) as wk, \
             tc.tile_pool(name="wk2", bufs=1) as wk2, \
             tc.tile_pool(name="ps_exc", bufs=4, space="PSUM") as ps_exc, \
             tc.tile_pool(name="ps_inh", bufs=4, space="PSUM") as ps_inh:

            cst_s = wpool.tile([P, _C_COLS], f32)
            # per-ob weight tiles, loaded in first-use order (ahead of need)
            wte_sb, wti_sb = [None] * OB, [None] * OB

            def load_weights(ob):
                # issued on the scalar (Activation) HW-DGE queue: parallel to
                # the sync queue carrying the critical x/ih/br tiles
                if ob >= OB or wte_sb[ob] is not None:
                    return
                we = wpool.tile([P, KE2, 2, P], f8, tag=f"wte{ob}")
                nc.scalar.dma_start(out=we, in_=wte[:, ob, :, :, :])
                wte_sb[ob] = we
                wi = wpool.tile([P, KI2, 2, P], f8, tag=f"wti{ob}")
                nc.scalar.dma_start(out=wi, in_=wti[:, ob, :, :, :])
                wti_sb[ob] = wi

            # critical lead-in: per-queue DMA streams run at only ~26 GB/s, so
            # split the first tiles into small pieces spread round-robin over
            # many queues; the first inh chain starts after ~190 KB lands.
            wi0 = wpool.tile([P, KI2, 2, P], f8, tag="wti0")
            nc.sync.dma_start(out=wi0[:, 0:2, :, :], in_=wti[:, 0, 0:2, :, :])
            wti_sb[0] = wi0

            xi_tiles = {}

            def load_nb(nb, split=False):
                if nb >= NB or nb in xi_tiles:
                    return
                xsa = xpool.tile([P, KQ2, 2, BSUB], f8, tag="xta")
                nc.sync.dma_start(out=xsa, in_=xt[nb, :, 0:KQ2, :, :])
                xsb = xpool.tile([P, KE2 - KQ2, 2, BSUB], f8, tag="xtb")
                KM = (KQ2 + KE2) // 2
                if split:
                    nc.sync.dma_start(out=xsb[:, 0:KM - KQ2, :, :],
                                      in_=xt[nb, :, KQ2:KM, :, :])
                    nc.sync.dma_start(out=xsb[:, KM - KQ2:, :, :],
                                      in_=xt[nb, :, KM:KE2, :, :])
                else:
                    nc.sync.dma_start(out=xsb, in_=xt[nb, :, KQ2:KE2, :, :])
                xs = (xsa, xsb)
                ihs = ipool.tile([P, KI2, 2, BSUB], f8, tag="iht")
                nc.sync.dma_start(out=ihs, in_=iht[nb, :, :, :, :])
                xi_tiles[nb] = (xs, ihs)

            ihs0 = ipool.tile([P, KI2, 2, BSUB], f8, tag="iht")
            nc.sync.dma_start(out=ihs0[:, 0:2, :, :], in_=iht[0, :, 0:2, :, :])
            nc.sync.dma_start(out=wi0[:, 2:KI2, :, :], in_=wti[:, 0, 2:KI2, :, :])
            nc.sync.dma_start(out=ihs0[:, 2:5, :, :], in_=iht[0, :, 2:5, :, :])
            nc.sync.dma_start(out=ihs0[:, 5:KI2, :, :], in_=iht[0, :, 5:KI2, :, :])
            we0 = wpool.tile([P, KE2, 2, P], f8, tag="wte0")
            nc.scalar.dma_start(out=we0[:, 0:8, :, :], in_=wte[:, 0, 0:8, :, :])
            nc.scalar.dma_start(out=we0[:, 8:KE2, :, :], in_=wte[:, 0, 8:KE2, :, :])
            wte_sb[0] = we0
            xsa0 = xpool.tile([P, KQ2, 2, BSUB], f8, tag="xta")
            nc.sync.dma_start(out=xsa0, in_=xt[0, :, 0:KQ2, :, :])
            xsb0 = xpool.tile([P, KE2 - KQ2, 2, BSUB], f8, tag="xtb")
            KM = (KQ2 + KE2) // 2
            nc.sync.dma_start(out=xsb0[:, 0:KM - KQ2, :, :],
                              in_=xt[0, :, KQ2:KM, :, :])
            nc.sync.dma_start(out=xsb0[:, KM - KQ2:, :, :],
                              in_=xt[0, :, KM:KE2, :, :])
            xi_tiles[0] = ((xsa0, xsb0), ihs0)
            nc.sync.dma_start(out=cst_s, in_=cst[:, :])

            for nb in range(NB):
                xt_s, iht_s = xi_tiles[nb]

                for ob in range(OB):
                    br_s = brpool.tile([P, BSUB, BF], f16, tag="br")
                    nc.sync.dma_start(out=br_s, in_=brt[nb, ob, :, :, :])
                    if nb == 0:
                        for ahead in (1, 2, 3, 4):
                            load_weights(ob + ahead)
                    if ob == OB - 5:
                        load_nb(nb + 1, split=True)

                    exc_ps = ps_exc.tile([P, BSUB], f32, tag="exc")
                    inh_ps = ps_inh.tile([P, BSUB], f32, tag="inh")

                    def emit_inh():
                        for k in range(KI2):
                            nc.tensor.matmul(
                                inh_ps, wti_sb[ob][:, k, :, :], iht_s[:, k, :, :],
                                start=(k == 0), stop=(k == KI2 - 1), perf_mode=DR)

                    def emit_exc():
                        xsa, xsb = xt_s
                        for k in range(KE2):
                            rhs = (xsa[:, k, :, :] if k < KQ2
                                   else xsb[:, k - KQ2, :, :])
                            nc.tensor.matmul(
                                exc_ps, wte_sb[ob][:, k, :, :], rhs,
                                start=(k == 0), stop=(k == KE2 - 1), perf_mode=DR)

                    last2 = nb == NB - 1 and ob >= OB - 2
                    if (nb == 0 and ob == 0) or last2:
                        # first iter: inh data lands first; last iters: finish
                        # inh early so the pointwise chain only waits on exc
                        emit_inh()
                        emit_exc()
                    else:
                        emit_exc()
                        emit_inh()

                    def pointwise(pool, c0, w, sfx):
                        cs = slice(c0, c0 + w)
                        ce_col = cst_s[:, _C_CE + ob: _C_CE + ob + 1]
                        ci_col = cst_s[:, _C_CI + ob: _C_CI + ob + 1]
                        # cur = sum_f br[:, cs, f]
                        cur = pool.tile([P, w], f32, tag="cur" + sfx)
                        if wb_ones:
                            # single strided reduce over the innermost f axis
                            nc.vector.tensor_reduce(
                                cur, br_s[:, cs, :], axis=AxX, op=add)
                        else:
                            nc.gpsimd.tensor_scalar_mul(
                                cur, br_s[:, cs, 0],
                                cst_s[:, _C_WB + ob * BF: _C_WB + ob * BF + 1])
                            for f in range(1, BF):
                                nxt = pool.tile([P, w], f32, tag=f"cur{f % 2}" + sfx)
                                nc.gpsimd.scalar_tensor_tensor(
                                    nxt, br_s[:, cs, f],
                                    cst_s[:, _C_WB + ob * BF + f: _C_WB + ob * BF + f + 1],
                                    cur, op0=mult, op1=add)
                                cur = nxt

                        # num = c_e * exc + cur   (one fused DVE op)
                        num = pool.tile([P, w], f32, tag="num" + sfx)
                        nc.vector.scalar_tensor_tensor(
                            num, exc_ps[:, cs], ce_col, cur, op0=mult, op1=add)
                        # exc1 = c_e * exc + (1 + cond) on ACT
                        exc1 = pool.tile([P, w], f32, tag="exc1" + sfx)
                        nc.scalar.activation(
                            exc1, exc_ps[:, cs], Identity,
                            bias=cst_s[:, _C_CP1 + ob: _C_CP1 + ob + 1],
                            scale=ce_col)
                        # den = c_i * inh + exc1  (DVE: gpsimd can't read PSUM)
                        den = pool.tile([P, w], f32, tag="den" + sfx)
                        nc.vector.scalar_tensor_tensor(
                            den, inh_ps[:, cs], ci_col, exc1, op0=mult, op1=add)
                        rden = pool.tile([P, w], f32, tag="rden" + sfx)
                        nc.vector.reciprocal_approx_fast(rden, den)
                        v = pool.tile([P, w], f32, tag="v" + sfx)
                        nc.gpsimd.tensor_mul(v, num, rden)
                        # r = relu(v - Vth); rate = (r * sqrt(alpha))^2
                        r = pool.tile([P, w], f32, tag="r" + sfx)
                        nc.scalar.activation(
                            r, v, Relu, bias=cst_s[:, _C_VTHN + ob: _C_VTHN + ob + 1])
                        ot = pool.tile([P, w], f16, tag="ot" + sfx)
                        nc.scalar.activation(
                            ot, r, Square, scale=cst_s[:, _C_SA + ob: _C_SA + ob + 1])
                        nc.sync.dma_start(out=outt[ob, :, nb, cs], in_=ot)

                    if nb == NB - 1 and ob == OB - 1:
                        # split the final chain so the kernel tail is shorter
                        q = BSUB // 4
                        for h in range(4):
                            pointwise(wk2, h * q, q, f"q{h}")
                    elif nb == NB - 1 and ob == OB - 2:
                        pointwise(wk2, 0, BSUB // 2, "h0")
                        pointwise(wk2, BSUB // 2, BSUB // 2, "h1")
                    else:
                        pointwise(wk, 0, BSUB, "")

    nc.compile()
    return nc


def _pruned_dense(pre_w, K):
    """Masked weight [out, in] fp32. Tie-break matches jax.lax.top_k:
    equal values -> lower index wins (stable sort)."""
    idx = np.argsort(-pre_w, axis=1, kind="stable")[:, :K]
    w = np.exp(pre_w.astype(np.float32))
    dense = np.zeros(pre_w.shape, dtype=np.float32)
    np.put_along_axis(dense, idx, np.take_along_axis(w, idx, axis=1), axis=1)
    return dense


def _quant_w(dense):
    """fp8 weights + per-output-row lsq correction c (dequant scale)."""
    q8 = dense.astype(F8)
    dq = q8.astype(np.float32)
    num = (dq * dense).sum(axis=1)
    den = (dq * dq).sum(axis=1)
    c = np.where(den > 0, num / np.maximum(den, 1e-30), 1.0).astype(np.float32)
    return q8, c


def kernel(x, inhibitory_input, branch_input, pre_w_exc, pre_w_inh,
           w_block, presigmoid_Vth, log_alpha_max):
    w_block = np.asarray(w_block, dtype=np.float32)
    wb_ones = bool(np.all(w_block == 1.0))
    key = ("nc", wb_ones)
    if key not in _CACHE:
        _CACHE[key] = _build_program(wb_ones)
    nc = _CACHE[key]

    x = np.ascontiguousarray(np.asarray(x, dtype=np.float32))
    inh = np.ascontiguousarray(np.asarray(inhibitory_input, dtype=np.float32))
    br = np.ascontiguousarray(np.asarray(branch_input, dtype=np.float32))
    pre_w_exc = np.asarray(pre_w_exc, dtype=np.float32)
    pre_w_inh = np.asarray(pre_w_inh, dtype=np.float32)
    presigmoid_Vth = np.asarray(presigmoid_Vth, dtype=np.float32)
    log_alpha_max = np.asarray(log_alpha_max, dtype=np.float32)

    # --- replicated operands -------------------------------------------------
    we8, ce = _quant_w(_pruned_dense(pre_w_exc, K_EXC))   # [OUT, EXC_IN] fp8
    wi8, ci = _quant_w(_pruned_dense(pre_w_inh, K_INH))   # [OUT, INH_IN] fp8
    # wte[p, ob, kk, j, o] = W8_exc[ob*P + o, (2*kk + j)*P + p]
    wte = np.ascontiguousarray(
        we8.T.reshape(KE2, 2, P, OB, P).transpose(2, 3, 0, 1, 4))
    wti = np.ascontiguousarray(
        wi8.T.reshape(KI2, 2, P, OB, P).transpose(2, 3, 0, 1, 4))

    cond = w_block.sum(axis=1, dtype=np.float32)              # [OUT]
    vth = (1.0 / (1.0 + np.exp(-presigmoid_Vth.astype(np.float64)))).astype(np.float32)
    sa = np.sqrt(np.exp(log_alpha_max.astype(np.float32)))
    cst = np.zeros((P, _C_COLS), dtype=np.float32)
    cst[:, _C_CP1:_C_CP1 + OB] = (1.0 + cond).reshape(OB, P).T
    cst[:, _C_VTHN:_C_VTHN + OB] = (-vth).reshape(OB, P).T
    cst[:, _C_SA:_C_SA + OB] = sa.reshape(OB, P).T
    cst[:, _C_CE:_C_CE + OB] = ce.reshape(OB, P).T
    cst[:, _C_CI:_C_CI + OB] = ci.reshape(OB, P).T
    cst[:, _C_WB:] = w_block.reshape(OB, P, BF).transpose(1, 0, 2).reshape(P, OB * BF)

    # --- per-core shards -----------------------------------------------------
    in_maps = []
    for c in range(NCORES):
        s = slice(c * BC, (c + 1) * BC)
        # xt[nb, p, kk, j, b] = x[c*BC + nb*BSUB + b, (2*kk + j)*P + p]
        xt = np.ascontiguousarray(
            x[s].astype(F8).reshape(NB, BSUB, KE2, 2, P).transpose(0, 4, 2, 3, 1))
        iht = np.ascontiguousarray(
            inh[s].astype(F8).reshape(NB, BSUB, KI2, 2, P).transpose(0, 4, 2, 3, 1))
        # brt[nb, ob, o, b, f] = branch[c*BC + nb*BSUB + b, (ob*P + o)*BF + f]
        brt = np.ascontiguousarray(
            br[s].astype(np.float16).reshape(NB, BSUB, OB, P, BF).transpose(0, 2, 3, 1, 4))
        in_maps.append({"wte": wte, "wti": wti, "cst": cst,
                        "xt": xt, "iht": iht, "brt": brt})

    res = run_bass_kernel_spmd(nc, in_maps, list(range(NCORES)), trace=TRACE)
    _CACHE["last"] = res

    out = np.empty((B, OUT), dtype=np.float32)
    for c in range(NCORES):
        # outt[ob, o, nb, b] -> out[c*BC + nb*BSUB + b, ob*P + o]
        ot = res.results[c]["outt"].astype(np.float32)
        out[c * BC:(c + 1) * BC] = ot.transpose(2, 3, 0, 1).reshape(BC, OUT)
    return out
